# revision 6
# baseline (speedup 1.0000x reference)
"""GQA causal attention (B=2,S=2048,D=2048,H=16,KV=4,HD=128) on 8 TRN2 NeuronCores.

Sharding: core c handles (batch b=c//4, kv-group g=c%4) — exactly 8 shards.
Each core computes q/k/v projections for its group's 4 query heads + 1 kv head,
RoPE, causal attention (512-wide q tiles, skipping fully-masked k blocks),
and a partial o-projection over its heads' slice of wo. Host sums the 4
group-partials per batch.

Device layouts are all "transposed" ([feature, seq]) so no on-device
transposes of activations are needed:
  - qT/kT: [hd, seq] with head-dim PERMUTED to [evens | odds] (the RoPE
    interleaved-pair rotation becomes two partition-aligned half-tiles).
    The permutation is folded into wq/wk columns host-side; scores are
    invariant since q and k are permuted consistently.
  - scoresT blocks: [sk=128, sq=512] = kT_blk.T @ qT_tile (one matmul, K=hd).
  - softmax without max-subtraction (|scores*scale| ~ O(10) in fp32 is safe);
    Z via ones-vector matmuls accumulated in PSUM; normalization by a
    gpsimd partition-broadcast reciprocal row.
  - AV: avT[hd, sq] += v_blk[sk, hd].T @ exp_blk[sk, sq]  (v natural layout,
    obtained from vT via 16 PE transposes).
  - o-proj: out[sq,d] += outT_h[:, sq_blk].T @ wo[h_blk, d]; partial written
    to HBM, summed on host across the 4 group cores per batch.

All matmuls run as float32r (full PE rate at N>=256, ~fp32 accuracy class).
"""

import numpy as np

B, S, D = 2, 2048, 2048
H, KV, HD = 16, 4, 128
GH = H // KV            # query heads per kv group (per core)
NCORES = 8
THETA = 10000.0
NEG = -1e9
SQT = 512               # q seq tile width
NSQ = S // SQT          # 4
NKB = S // 128          # 16 k blocks
NCH = D // 128          # 16 contraction chunks
NOFF = SQT // 128       # 4 diagonal offsets

SCALE = float(HD) ** -0.5

# Exposed for the dev harness (test.py) to read profiling results.
last_results = None


def _build_program():
    from contextlib import ExitStack

    import concourse.bass as bass
    import concourse.tile as tile
    from concourse import bacc, mybir
    from concourse.masks import make_identity

    f32 = mybir.dt.float32
    f32r = mybir.dt.float32r
    EXP = mybir.ActivationFunctionType.Exp

    def r(ap):
        return ap.bitcast(f32r)

    nc = bacc.Bacc("TRN2", target_bir_lowering=False, debug=False,
                   num_devices=NCORES)

    xT_d = nc.dram_tensor("xT", [D, S], f32r, kind="ExternalInput")
    wq_d = nc.dram_tensor("wqp", [D, GH * HD], f32r, kind="ExternalInput")
    wk_d = nc.dram_tensor("wkp", [D, HD], f32r, kind="ExternalInput")
    wv_d = nc.dram_tensor("wvg", [D, HD], f32r, kind="ExternalInput")
    wo_d = nc.dram_tensor("wog", [GH * HD, D], f32r, kind="ExternalInput")
    cos_d = nc.dram_tensor("cos2", [HD, S], f32, kind="ExternalInput")
    sin_d = nc.dram_tensor("sinS", [HD, S], f32, kind="ExternalInput")
    msk_d = nc.dram_tensor("m01", [128, NOFF, SQT], f32, kind="ExternalInput")
    one_d = nc.dram_tensor("ones1", [128, 1], f32r, kind="ExternalInput")
    out_d = nc.dram_tensor("out", [S, D], f32, kind="ExternalOutput")

    xT_v = xT_d.ap().rearrange("(c p) s -> p c s", p=128)     # [128,16,2048]
    wq_v = wq_d.ap().rearrange("(c p) m -> p c m", p=128)     # [128,16,512]
    wk_v = wk_d.ap().rearrange("(c p) m -> p c m", p=128)     # [128,16,128]
    wv_v = wv_d.ap().rearrange("(c p) m -> p c m", p=128)
    wo_v = wo_d.ap().rearrange("(h p) d -> p h d", p=128)     # [128,4,2048]
    out_v = out_d.ap().rearrange("(m p) d -> p m d", p=128)   # [128,16,2048]

    with tile.TileContext(nc) as tc, ExitStack() as ctx:
        persist = ctx.enter_context(tc.tile_pool(name="persist", bufs=1))

        qT = [persist.tile([128, S], f32r, name=f"qT{h}") for h in range(GH)]
        kT = persist.tile([128, S], f32r, name="kT")
        vn = persist.tile([128, NKB, HD], f32r, name="vn")
        outT = [persist.tile([128, S], f32r, name=f"outT{h}") for h in range(GH)]
        cos2 = persist.tile([128, S], f32, name="cos2")
        sinS = persist.tile([128, S], f32, name="sinS")
        m01 = persist.tile([128, NOFF, SQT], f32, name="m01")
        ones = persist.tile([128, 1], f32r, name="ones")
        ident = persist.tile([128, 128], f32, name="ident")

        nc.sync.dma_start(cos2[:], cos_d[:])
        nc.sync.dma_start(sinS[:], sin_d[:])
        nc.sync.dma_start(m01[:], msk_d[:])
        nc.sync.dma_start(ones[:], one_d[:])
        make_identity(nc, ident[:])

        # ---------------- Phase 1: projections + RoPE + v ----------------
        with (
            tc.tile_pool(name="w1", bufs=1) as w1p,
            tc.tile_pool(name="xa", bufs=4) as xap,
            tc.tile_pool(name="raw", bufs=2) as rawp,
            tc.tile_pool(name="rope", bufs=2) as ropep,
            tc.tile_pool(name="ps1", bufs=1, space="PSUM") as ps1,
            tc.tile_pool(name="tps", bufs=2, space="PSUM") as tps,
        ):
            wq_sb = w1p.tile([128, NCH, GH * HD], f32r)
            wk_sb = w1p.tile([128, NCH, HD], f32r)
            wv_sb = w1p.tile([128, NCH, HD], f32r)
            vT_tmp = w1p.tile([128, S], f32)
            nc.sync.dma_start(wq_sb[:], wq_v)
            nc.sync.dma_start(wk_sb[:], wk_v)
            nc.sync.dma_start(wv_sb[:], wv_v)

            def rope(raw, dst, t):
                """dst[:, t-tile] = rope(raw) in the [evens|odds] layout."""
                sl = np.s_[:, t * SQT:(t + 1) * SQT]
                tmp = ropep.tile([128, SQT], f32, tag="ropetmp")
                swp = ropep.tile([128, SQT], f32, tag="ropeswp")
                nc.vector.tensor_mul(tmp[:], raw[:], cos2[sl])
                # swp[0:64] = odd*(-sin), swp[64:128] = even*(+sin); sinS is
                # stored [+sin | -sin] so each mul's two INPUTS share a base
                # partition (walrus requires that); only the output crosses.
                nc.vector.tensor_mul(swp[0:64, :], raw[64:128, :],
                                     sinS[sl][64:128, :])
                nc.vector.tensor_mul(swp[64:128, :], raw[0:64, :],
                                     sinS[sl][0:64, :])
                nc.vector.tensor_add(dst[sl], tmp[:], swp[:])

            for t in range(NSQ):
                ssl = np.s_[t * SQT:(t + 1) * SQT]
                q_ps = [ps1.tile([128, SQT], f32, tag=f"qps{h}", name=f"qps{h}")
                        for h in range(GH)]
                k_ps = ps1.tile([128, SQT], f32, tag="kps")
                v_ps = ps1.tile([128, SQT], f32, tag="vps")
                for c in range(NCH):
                    xc = xap.tile([128, SQT], f32r, tag="xc")
                    nc.sync.dma_start(xc[:], xT_v[:, c, ssl])
                    st, sp = c == 0, c == NCH - 1
                    for h in range(GH):
                        nc.tensor.matmul(
                            q_ps[h][:], r(wq_sb[:, c, h * HD:(h + 1) * HD]),
                            r(xc[:]), start=st, stop=sp)
                    nc.tensor.matmul(k_ps[:], r(wk_sb[:, c, :]), r(xc[:]),
                                     start=st, stop=sp)
                    nc.tensor.matmul(v_ps[:], r(wv_sb[:, c, :]), r(xc[:]),
                                     start=st, stop=sp)
                # move psum -> sbuf on ACT (frees banks fast), rope on DVE
                for h in range(GH):
                    qraw = rawp.tile([128, SQT], f32, tag=f"qraw{h}")
                    nc.scalar.copy(qraw[:], q_ps[h][:])
                    rope(qraw, qT[h], t)
                kraw = rawp.tile([128, SQT], f32, tag="kraw")
                nc.scalar.copy(kraw[:], k_ps[:])
                rope(kraw, kT, t)
                nc.scalar.copy(vT_tmp[:, ssl], v_ps[:])

            # v natural blocks via PE transpose
            for j in range(NKB):
                t_ps = tps.tile([128, 128], f32, tag="tps")
                nc.tensor.transpose(t_ps[:], vT_tmp[:, j * 128:(j + 1) * 128],
                                    ident[:])
                nc.vector.tensor_copy(vn[:, j, :], t_ps[:])

        # ---------------- Phase 2: attention ----------------
        with (
            tc.tile_pool(name="ex", bufs=6) as exp_p,
            tc.tile_pool(name="nrm", bufs=4) as nrm_p,
            tc.tile_pool(name="sps", bufs=3, space="PSUM") as sps_p,
            tc.tile_pool(name="avps", bufs=2, space="PSUM") as avp_p,
            tc.tile_pool(name="zps", bufs=2, space="PSUM") as zp_p,
        ):
            for h in range(GH):
                for t in range(NSQ):
                    qsl = np.s_[:, t * SQT:(t + 1) * SQT]
                    nblk = NOFF * (t + 1)
                    av_ps = avp_p.tile([HD, SQT], f32, tag="av")
                    z_ps = zp_p.tile([1, SQT], f32, tag="z")
                    for j in range(nblk):
                        s_ps = sps_p.tile([128, SQT], f32, tag="s")
                        nc.tensor.matmul(
                            s_ps[:], r(kT[:, j * 128:(j + 1) * 128]),
                            r(qT[h][qsl]))
                        e = exp_p.tile([128, SQT], f32r, tag="e")
                        nc.scalar.activation(e[:], s_ps[:], EXP, scale=SCALE)
                        o = j - NOFF * t
                        if o >= 0:  # diagonal region: zero out sk > sq
                            nc.vector.tensor_mul(e[:], e[:], m01[:, o, :])
                        st, sp = j == 0, j == nblk - 1
                        nc.tensor.matmul(av_ps[:], r(vn[:, j, :]), r(e[:]),
                                         start=st, stop=sp)
                        nc.tensor.matmul(z_ps[:], r(ones[:]), r(e[:]),
                                         start=st, stop=sp)
                    zrec = nrm_p.tile([1, SQT], f32, tag="zrec")
                    nc.vector.reciprocal(zrec[:], z_ps[:])
                    zb = nrm_p.tile([128, SQT], f32, tag="zb")
                    nc.gpsimd.partition_broadcast(zb[:], zrec[:])
                    nc.vector.tensor_mul(outT[h][qsl], av_ps[:], zb[:])

        # ---------------- Phase 3: o-projection ----------------
        with (
            tc.tile_pool(name="wo", bufs=1) as wop,
            tc.tile_pool(name="osb", bufs=4) as osb_p,
            tc.tile_pool(name="ops", bufs=4, space="PSUM") as ops_p,
        ):
            wo_sb = wop.tile([128, GH, D], f32r)
            nc.sync.dma_start(wo_sb[:], wo_v)
            for m in range(S // 128):
                for jd in range(D // SQT):
                    dsl = np.s_[jd * SQT:(jd + 1) * SQT]
                    o_ps = ops_p.tile([128, SQT], f32, tag="o")
                    for h in range(GH):
                        nc.tensor.matmul(
                            o_ps[:], r(outT[h][:, m * 128:(m + 1) * 128]),
                            r(wo_sb[:, h, dsl]),
                            start=(h == 0), stop=(h == GH - 1))
                    ob = osb_p.tile([128, SQT], f32, tag="ob")
                    if (m + jd) % 2 == 0:
                        nc.vector.tensor_copy(ob[:], o_ps[:])
                    else:
                        nc.scalar.copy(ob[:], o_ps[:])
                    nc.sync.dma_start(out_v[:, m, dsl], ob[:])

    nc.compile()
    return nc


_prog = None


def _host_inputs(x, wq, wk, wv, wo):
    """Per-core input maps (core c -> batch c//KV... see module docstring)."""
    perm = np.concatenate([np.arange(0, HD, 2), np.arange(1, HD, 2)])
    wq_p = np.ascontiguousarray(
        wq.reshape(D, H, HD)[:, :, perm].reshape(D, H * HD))
    wk_p = np.ascontiguousarray(
        wk.reshape(D, KV, HD)[:, :, perm].reshape(D, KV * HD))

    inv_freq = 1.0 / (THETA ** (np.arange(0, HD, 2, dtype=np.float64) / HD))
    freqs = np.outer(np.arange(S, dtype=np.float64), inv_freq)   # [S, 64]
    cosT = np.cos(freqs).T.astype(np.float32)                    # [64, S]
    sinT = np.sin(freqs).T.astype(np.float32)
    cos2 = np.ascontiguousarray(np.concatenate([cosT, cosT], 0))
    sinS = np.ascontiguousarray(np.concatenate([sinT, -sinT], 0))

    sk = np.arange(128)[:, None]
    sq = np.arange(SQT)[None, :]
    m01 = np.stack([(sk <= sq - 128 * o).astype(np.float32)
                    for o in range(NOFF)], axis=1)               # [128,4,512]
    m01 = np.ascontiguousarray(m01)

    in_maps = []
    for c in range(NCORES):
        b, g = c // KV, c % KV
        in_maps.append({
            "xT": np.ascontiguousarray(x[b].T),
            "wqp": np.ascontiguousarray(wq_p[:, g * GH * HD:(g + 1) * GH * HD]),
            "wkp": np.ascontiguousarray(wk_p[:, g * HD:(g + 1) * HD]),
            "wvg": np.ascontiguousarray(wv[:, g * HD:(g + 1) * HD]),
            "wog": np.ascontiguousarray(wo[g * GH * HD:(g + 1) * GH * HD, :]),
            "cos2": cos2,
            "sinS": sinS,
            "m01": m01,
            "ones1": np.ones((128, 1), np.float32),
        })
    return in_maps


def _numpy_reference(x, mask, wq, wk, wv, wo):
    """Pure-numpy fallback for inputs this kernel isn't specialized for."""
    b, s, _ = x.shape
    q = (x @ wq).reshape(b, s, H, HD)
    k = (x @ wk).reshape(b, s, KV, HD)
    v = (x @ wv).reshape(b, s, KV, HD)
    inv_freq = 1.0 / (THETA ** (np.arange(0, HD, 2, dtype=np.float32) / HD))
    t = np.arange(s, dtype=np.float32)
    freqs = np.outer(t, inv_freq)
    cos = np.cos(freqs)[:, None, :]
    sin = np.sin(freqs)[:, None, :]

    def rot(a):
        bb, ss, nh, hd = a.shape
        a = a.reshape(bb, ss, nh, hd // 2, 2)
        a0, a1 = a[..., 0], a[..., 1]
        out = np.stack([a0 * cos - a1 * sin, a0 * sin + a1 * cos], axis=-1)
        return out.reshape(bb, ss, nh, hd)

    q, k = rot(q), rot(k)
    rep = H // KV
    k = np.repeat(k, rep, axis=2)
    v = np.repeat(v, rep, axis=2)
    q, k, v = (a.transpose(0, 2, 1, 3) for a in (q, k, v))
    scores = np.einsum("bhqd,bhkd->bhqk", q, k) * SCALE + mask
    scores = scores - scores.max(axis=-1, keepdims=True)
    e = np.exp(scores)
    attn = e / e.sum(axis=-1, keepdims=True)
    out = np.einsum("bhqk,bhkd->bhqd", attn, v)
    out = out.transpose(0, 2, 1, 3).reshape(b, s, H * HD)
    return (out @ wo).astype(np.float32)


def kernel(x, mask, wq, wk, wv, wo):
    global _prog, last_results
    x = np.asarray(x, np.float32)
    mask = np.asarray(mask, np.float32)
    wq, wk, wv, wo = (np.asarray(a, np.float32) for a in (wq, wk, wv, wo))

    causal = np.where(np.tril(np.ones((S, S), bool)), 0.0, NEG).astype(np.float32)
    if (x.shape != (B, S, D) or mask.shape != (S, S)
            or not np.array_equal(mask, causal)):
        return _numpy_reference(x, mask, wq, wk, wv, wo)

    from concourse import bass_utils

    if _prog is None:
        _prog = _build_program()

    in_maps = _host_inputs(x, wq, wk, wv, wo)
    last_results = bass_utils.run_bass_kernel_spmd(
        _prog, in_maps, core_ids=list(range(NCORES)))
    parts = [res["out"] for res in last_results.results]
    out = np.empty((B, S, D), np.float32)
    for b in range(B):
        out[b] = parts[KV * b] + parts[KV * b + 1] + parts[KV * b + 2] + parts[KV * b + 3]
    return out


# revision 12
# speedup vs baseline: 1.1297x; 1.1297x over previous
"""GQA causal attention (B=2,S=2048,D=2048,H=16,KV=4,HD=128) on 8 TRN2 NeuronCores.

Sharding: core c handles (batch b=c//4, kv-group g=c%4) — exactly 8 shards.
Each core computes q/k/v projections for its group's 4 query heads + 1 kv head,
RoPE, causal attention (512-wide q tiles, skipping fully-masked k blocks),
and a partial o-projection over its heads' slice of wo. Host sums the 4
group-partials per batch.

Device layouts are all "transposed" ([feature, seq]) so no on-device
transposes of activations are needed:
  - qT/kT: [hd, seq] with head-dim PERMUTED to [evens | odds] (the RoPE
    interleaved-pair rotation becomes two partition-aligned half-tiles).
    The permutation is folded into wq/wk columns host-side; scores are
    invariant since q and k are permuted consistently.
  - scoresT blocks: [sk=128, sq<=512] = kT_blk.T @ qT_tile (one matmul, K=hd).
    Diagonal-region blocks are narrowed to their causally-live sq range
    (min N=256 to stay at full float32r rate).
  - softmax without max-subtraction (|scores*scale| ~ O(10) in fp32 is safe);
    Z via ones-vector matmuls accumulated in PSUM; reciprocal batched [4,512]
    per q-tile; normalization via gpsimd partition-broadcast + DVE multiply.
  - AV: avT[hd, sq] += v_blk[sk, hd].T @ exp_blk[sk, sq]  (v natural layout,
    from vT via PE transposes, done per seq-tile so attention can start early).
  - o-proj interleaved per q-tile: out[sq,d] += outT_h[:, sq_blk].T @ wo[h_blk, d];
    partial written to HBM, summed on host across the 4 group cores per batch.

All matmuls run as float32r (full PE rate at N>=256, ~fp32 accuracy class).
"""

import numpy as np

B, S, D = 2, 2048, 2048
H, KV, HD = 16, 4, 128
GH = H // KV            # query heads per kv group (per core)
NCORES = 8
THETA = 10000.0
NEG = -1e9
SQT = 512               # q seq tile width
NSQ = S // SQT          # 4
NKB = S // 128          # 16 k blocks
NCH = D // 128          # 16 contraction chunks
NOFF = SQT // 128       # 4 diagonal offsets

SCALE = float(HD) ** -0.5

# Narrowed column start for diagonal-region block at offset o: the block is
# causally dead below sq_local = 128*o; keep N >= 256 for full f32r rate.
NARROW = [0, 128, 256, 256]
# Columns needing the 0/1 mask multiply (tri block + any dead cols kept wide).
MASKW = [(0, 128), (128, 256), (256, 384), (256, 512)]

# Exposed for the dev harness (test.py) to read profiling results.
last_results = None


def _build_program():
    from contextlib import ExitStack

    import concourse.tile as tile
    from concourse import bacc, mybir
    from concourse.masks import make_identity

    f32 = mybir.dt.float32
    f32r = mybir.dt.float32r
    EXP = mybir.ActivationFunctionType.Exp

    def r(ap):
        return ap.bitcast(f32r)

    nc = bacc.Bacc("TRN2", target_bir_lowering=False, debug=False,
                   num_devices=NCORES)

    xT_d = nc.dram_tensor("xT", [D, S], f32r, kind="ExternalInput")
    wq_d = nc.dram_tensor("wqp", [D, GH * HD], f32r, kind="ExternalInput")
    wk_d = nc.dram_tensor("wkp", [D, HD], f32r, kind="ExternalInput")
    wv_d = nc.dram_tensor("wvg", [D, HD], f32r, kind="ExternalInput")
    wo_d = nc.dram_tensor("wog", [GH * HD, D], f32r, kind="ExternalInput")
    cos_d = nc.dram_tensor("cos2", [HD, S], f32, kind="ExternalInput")
    sin_d = nc.dram_tensor("sinS", [HD, S], f32, kind="ExternalInput")
    msk_d = nc.dram_tensor("m01", [128, NOFF, SQT], f32, kind="ExternalInput")
    one_d = nc.dram_tensor("ones1", [128, 1], f32r, kind="ExternalInput")
    out_d = nc.dram_tensor("out", [S, D], f32, kind="ExternalOutput")

    xT_v = xT_d.ap().rearrange("(c p) s -> p c s", p=128)     # [128,16,2048]
    wq_v = wq_d.ap().rearrange("(c p) m -> p c m", p=128)     # [128,16,512]
    wk_v = wk_d.ap().rearrange("(c p) m -> p c m", p=128)     # [128,16,128]
    wv_v = wv_d.ap().rearrange("(c p) m -> p c m", p=128)
    wo_v = wo_d.ap().rearrange("(h p) d -> p h d", p=128)     # [128,4,2048]
    out_v = out_d.ap().rearrange("(m p) d -> p m d", p=128)   # [128,16,2048]

    with tile.TileContext(nc) as tc, ExitStack() as ctx:
        persist = ctx.enter_context(tc.tile_pool(name="persist", bufs=1))

        qT = [persist.tile([128, S], f32r, name=f"qT{h}") for h in range(GH)]
        kT = persist.tile([128, S], f32r, name="kT")
        vn = persist.tile([128, NKB, HD], f32r, name="vn")
        outT = [persist.tile([128, S], f32r, name=f"outT{h}") for h in range(GH)]
        cos2 = persist.tile([128, S], f32, name="cos2")
        sinS = persist.tile([128, S], f32, name="sinS")
        m01 = persist.tile([128, NOFF, SQT], f32, name="m01")
        ones = persist.tile([128, 1], f32r, name="ones")
        ident = persist.tile([128, 128], f32, name="ident")

        # aux loads on the ACT DMA queue; bulk data goes via Sync so the
        # first weight/x tiles are in flight immediately
        nc.scalar.dma_start(cos2[:], cos_d[:])
        nc.scalar.dma_start(sinS[:], sin_d[:])
        nc.scalar.dma_start(m01[:], msk_d[:])
        nc.scalar.dma_start(ones[:], one_d[:])
        make_identity(nc, ident[:])

        # ---------------- Phase 1: projections + RoPE + v ----------------
        with (
            tc.tile_pool(name="w1", bufs=1) as w1p,
            tc.tile_pool(name="xa", bufs=4) as xap,
            tc.tile_pool(name="raw", bufs=2) as rawp,
            tc.tile_pool(name="rope", bufs=2) as ropep,
            tc.tile_pool(name="ps1", bufs=1, space="PSUM") as ps1,
            tc.tile_pool(name="tps", bufs=2, space="PSUM") as tps,
        ):
            wq_sb = w1p.tile([128, NCH, GH * HD], f32r)
            wk_sb = w1p.tile([128, NCH, HD], f32r)
            wv_sb = w1p.tile([128, NCH, HD], f32r)
            vT_tmp = w1p.tile([128, S], f32)
            nc.sync.dma_start(wk_sb[:], wk_v)
            nc.sync.dma_start(wv_sb[:], wv_v)
            for cq in range(4):  # split so early q matmuls aren't gated
                nc.sync.dma_start(wq_sb[:, 4 * cq:4 * cq + 4, :],
                                  wq_v[:, 4 * cq:4 * cq + 4, :])

            def rope(raw, dst, t):
                """dst[:, t-tile] = rope(raw) in the [evens|odds] layout."""
                sl = np.s_[:, t * SQT:(t + 1) * SQT]
                tmp = ropep.tile([128, SQT], f32, tag="ropetmp", name="tmp")
                swp = ropep.tile([128, SQT], f32, tag="ropeswp", name="swp")
                nc.vector.tensor_mul(tmp[:], raw[:], cos2[sl])
                # swp[0:64] = odd*(-sin), swp[64:128] = even*(+sin); sinS is
                # stored [+sin | -sin] so each mul's two INPUTS share a base
                # partition (walrus requires that); only the output crosses.
                nc.vector.tensor_mul(swp[0:64, :], raw[64:128, :],
                                     sinS[sl][64:128, :])
                nc.vector.tensor_mul(swp[64:128, :], raw[0:64, :],
                                     sinS[sl][0:64, :])
                nc.vector.tensor_add(dst[sl], tmp[:], swp[:])

            for t in range(NSQ):
                ssl = np.s_[t * SQT:(t + 1) * SQT]
                q_ps = [ps1.tile([128, SQT], f32, tag=f"qps{h}", name=f"qps{h}")
                        for h in range(GH)]
                k_ps = ps1.tile([128, SQT], f32, tag="kps", name="k_ps")
                v_ps = ps1.tile([128, SQT], f32, tag="vps", name="v_ps")
                for c in range(NCH):
                    xc = xap.tile([128, SQT], f32r, tag="xc", name="xc")
                    nc.sync.dma_start(xc[:], xT_v[:, c, ssl])
                    st, sp = c == 0, c == NCH - 1
                    for h in range(GH):
                        nc.tensor.matmul(
                            q_ps[h][:], r(wq_sb[:, c, h * HD:(h + 1) * HD]),
                            r(xc[:]), start=st, stop=sp)
                    nc.tensor.matmul(k_ps[:], r(wk_sb[:, c, :]), r(xc[:]),
                                     start=st, stop=sp)
                    nc.tensor.matmul(v_ps[:], r(wv_sb[:, c, :]), r(xc[:]),
                                     start=st, stop=sp)
                # move psum -> sbuf on ACT (frees banks fast), rope on DVE
                for h in range(GH):
                    qraw = rawp.tile([128, SQT], f32, tag=f"qraw{h}",
                                     name=f"qraw{h}")
                    nc.scalar.copy(qraw[:], q_ps[h][:])
                    rope(qraw, qT[h], t)
                kraw = rawp.tile([128, SQT], f32, tag="kraw", name="kraw")
                nc.scalar.copy(kraw[:], k_ps[:])
                rope(kraw, kT, t)
                nc.scalar.copy(vT_tmp[:, ssl], v_ps[:])
                # v natural blocks for this seq-tile (attention needs them
                # as soon as the t=0 slices exist)
                for j in range(NOFF * t, NOFF * (t + 1)):
                    t_ps = tps.tile([128, 128], f32, tag="tps", name="t_ps")
                    nc.tensor.transpose(
                        t_ps[:], vT_tmp[:, j * 128:(j + 1) * 128], ident[:])
                    nc.vector.tensor_copy(vn[:, j, :], t_ps[:])

        # -------- Phase 2: attention + o-projection, q-tile major --------
        with (
            tc.tile_pool(name="wo", bufs=1) as wop,
            tc.tile_pool(name="ex", bufs=6) as exp_p,
            tc.tile_pool(name="nrm", bufs=2) as nrm_p,
            tc.tile_pool(name="osb", bufs=4) as osb_p,
            tc.tile_pool(name="sps", bufs=3, space="PSUM") as sps_p,
            tc.tile_pool(name="avps", bufs=2, space="PSUM") as avp_p,
            tc.tile_pool(name="zps", bufs=1, space="PSUM") as zp_p,
            tc.tile_pool(name="ops", bufs=2, space="PSUM") as ops_p,
        ):
            wo_sb = wop.tile([128, GH, D], f32r)
            nc.sync.dma_start(wo_sb[:], wo_v)

            for t in range(NSQ):
                qsl = np.s_[:, t * SQT:(t + 1) * SQT]
                nblk = NOFF * (t + 1)
                # head h's Z row lives at partition 32h (engine APs may only
                # start at partitions 0/32/64/96); unused rows memset to 1.0
                # so the batched reciprocal stays finite.
                zall = nrm_p.tile([128, SQT], f32, tag="zall", name="zall")
                nc.gpsimd.memset(zall[:], 1.0)
                av_list = []
                for h in range(GH):
                    av_ps = avp_p.tile([HD, SQT], f32, tag="av", name="av_ps")
                    z_ps = zp_p.tile([1, SQT], f32, tag="z", name="z_ps")
                    for j in range(nblk):
                        o = j - NOFF * t
                        lo = NARROW[o] if o >= 0 else 0
                        csl = np.s_[:, lo:SQT]
                        s_ps = sps_p.tile([128, SQT], f32, tag="s", name="s_ps")
                        nc.tensor.matmul(
                            s_ps[csl], r(kT[:, j * 128:(j + 1) * 128]),
                            r(qT[h][qsl][csl]))
                        e = exp_p.tile([128, SQT], f32r, tag="e", name="e")
                        nc.scalar.activation(e[csl], s_ps[csl], EXP,
                                             scale=SCALE)
                        if o >= 0:  # mask the causally-dead part
                            ma, mb = MASKW[o]
                            msl = np.s_[:, ma:mb]
                            nc.vector.tensor_mul(e[msl], e[msl],
                                                 m01[:, o, ma:mb])
                        st, sp = j == 0, j == nblk - 1
                        nc.tensor.matmul(av_ps[csl], r(vn[:, j, :]), r(e[csl]),
                                         start=st, stop=sp)
                        nc.tensor.matmul(z_ps[csl], r(ones[:]), r(e[csl]),
                                         start=st, stop=sp)
                    nc.vector.tensor_copy(zall[32 * h:32 * h + 1, :], z_ps[:])
                    # free the AV psum bank fast: unnormalized copy on ACT
                    avo = nrm_p.tile([HD, SQT], f32, tag=f"avo{h}",
                                     name=f"avo{h}", bufs=1)
                    nc.scalar.copy(avo[:], av_ps[:])
                    av_list.append(avo)
                zrec = nrm_p.tile([128, SQT], f32, tag="zrec", name="zrec")
                nc.vector.reciprocal(zrec[:], zall[:])
                for h in range(GH):
                    # stage to base partition 0: the HW broadcast ucode does
                    # not honor a nonzero AP base partition
                    z1 = nrm_p.tile([1, SQT], f32, tag="z1", name="z1")
                    nc.vector.tensor_copy(z1[:], zrec[32 * h:32 * h + 1, :])
                    zb = nrm_p.tile([128, SQT], f32, tag="zb", name="zb")
                    nc.gpsimd.partition_broadcast(zb[:], z1[:])
                    nc.vector.tensor_mul(outT[h][qsl], av_list[h][:], zb[:])

                # o-projection for this q-tile's four 128-row blocks
                for m in range(NOFF * t, NOFF * (t + 1)):
                    for jd in range(D // SQT):
                        dsl = np.s_[jd * SQT:(jd + 1) * SQT]
                        o_ps = ops_p.tile([128, SQT], f32, tag="o", name="o_ps")
                        for h in range(GH):
                            nc.tensor.matmul(
                                o_ps[:], r(outT[h][:, m * 128:(m + 1) * 128]),
                                r(wo_sb[:, h, dsl]),
                                start=(h == 0), stop=(h == GH - 1))
                        ob = osb_p.tile([128, SQT], f32, tag="ob", name="ob")
                        if (m + jd) % 2 == 0:
                            nc.vector.tensor_copy(ob[:], o_ps[:])
                        else:
                            nc.scalar.copy(ob[:], o_ps[:])
                        nc.sync.dma_start(out_v[:, m, dsl], ob[:])

    nc.compile()
    return nc


_prog = None


def _host_inputs(x, wq, wk, wv, wo):
    """Per-core input maps (core c -> batch c//KV, kv-group c%KV)."""
    perm = np.concatenate([np.arange(0, HD, 2), np.arange(1, HD, 2)])
    wq_p = np.ascontiguousarray(
        wq.reshape(D, H, HD)[:, :, perm].reshape(D, H * HD))
    wk_p = np.ascontiguousarray(
        wk.reshape(D, KV, HD)[:, :, perm].reshape(D, KV * HD))

    inv_freq = 1.0 / (THETA ** (np.arange(0, HD, 2, dtype=np.float64) / HD))
    freqs = np.outer(np.arange(S, dtype=np.float64), inv_freq)   # [S, 64]
    cosT = np.cos(freqs).T.astype(np.float32)                    # [64, S]
    sinT = np.sin(freqs).T.astype(np.float32)
    cos2 = np.ascontiguousarray(np.concatenate([cosT, cosT], 0))
    sinS = np.ascontiguousarray(np.concatenate([sinT, -sinT], 0))

    sk = np.arange(128)[:, None]
    sq = np.arange(SQT)[None, :]
    m01 = np.stack([(sk <= sq - 128 * o).astype(np.float32)
                    for o in range(NOFF)], axis=1)               # [128,4,512]
    m01 = np.ascontiguousarray(m01)

    in_maps = []
    for c in range(NCORES):
        b, g = c // KV, c % KV
        in_maps.append({
            "xT": np.ascontiguousarray(x[b].T),
            "wqp": np.ascontiguousarray(wq_p[:, g * GH * HD:(g + 1) * GH * HD]),
            "wkp": np.ascontiguousarray(wk_p[:, g * HD:(g + 1) * HD]),
            "wvg": np.ascontiguousarray(wv[:, g * HD:(g + 1) * HD]),
            "wog": np.ascontiguousarray(wo[g * GH * HD:(g + 1) * GH * HD, :]),
            "cos2": cos2,
            "sinS": sinS,
            "m01": m01,
            "ones1": np.ones((128, 1), np.float32),
        })
    return in_maps


def _numpy_reference(x, mask, wq, wk, wv, wo):
    """Pure-numpy fallback for inputs this kernel isn't specialized for."""
    b, s, _ = x.shape
    q = (x @ wq).reshape(b, s, H, HD)
    k = (x @ wk).reshape(b, s, KV, HD)
    v = (x @ wv).reshape(b, s, KV, HD)
    inv_freq = 1.0 / (THETA ** (np.arange(0, HD, 2, dtype=np.float32) / HD))
    t = np.arange(s, dtype=np.float32)
    freqs = np.outer(t, inv_freq)
    cos = np.cos(freqs)[:, None, :]
    sin = np.sin(freqs)[:, None, :]

    def rot(a):
        bb, ss, nh, hd = a.shape
        a = a.reshape(bb, ss, nh, hd // 2, 2)
        a0, a1 = a[..., 0], a[..., 1]
        out = np.stack([a0 * cos - a1 * sin, a0 * sin + a1 * cos], axis=-1)
        return out.reshape(bb, ss, nh, hd)

    q, k = rot(q), rot(k)
    rep = H // KV
    k = np.repeat(k, rep, axis=2)
    v = np.repeat(v, rep, axis=2)
    q, k, v = (a.transpose(0, 2, 1, 3) for a in (q, k, v))
    scores = np.einsum("bhqd,bhkd->bhqk", q, k) * SCALE + mask
    scores = scores - scores.max(axis=-1, keepdims=True)
    e = np.exp(scores)
    attn = e / e.sum(axis=-1, keepdims=True)
    out = np.einsum("bhqk,bhkd->bhqd", attn, v)
    out = out.transpose(0, 2, 1, 3).reshape(b, s, H * HD)
    return (out @ wo).astype(np.float32)


def kernel(x, mask, wq, wk, wv, wo):
    global _prog, last_results
    x = np.asarray(x, np.float32)
    mask = np.asarray(mask, np.float32)
    wq, wk, wv, wo = (np.asarray(a, np.float32) for a in (wq, wk, wv, wo))

    causal = np.where(np.tril(np.ones((S, S), bool)), 0.0, NEG).astype(np.float32)
    if (x.shape != (B, S, D) or mask.shape != (S, S)
            or not np.array_equal(mask, causal)):
        return _numpy_reference(x, mask, wq, wk, wv, wo)

    from concourse import bass_utils

    if _prog is None:
        _prog = _build_program()

    in_maps = _host_inputs(x, wq, wk, wv, wo)
    last_results = bass_utils.run_bass_kernel_spmd(
        _prog, in_maps, core_ids=list(range(NCORES)))
    parts = [res["out"] for res in last_results.results]
    out = np.empty((B, S, D), np.float32)
    for b in range(B):
        out[b] = parts[KV * b] + parts[KV * b + 1] + parts[KV * b + 2] + parts[KV * b + 3]
    return out


# revision 14
# speedup vs baseline: 1.1678x; 1.0337x over previous
"""GQA causal attention (B=2,S=2048,D=2048,H=16,KV=4,HD=128) on 8 TRN2 NeuronCores.

Sharding: core c handles (batch b=c//4, kv-group g=c%4) — exactly 8 shards.
Each core computes q/k/v projections for its group's 4 query heads + 1 kv head,
RoPE, causal attention (512-wide q tiles, skipping fully-masked k blocks),
and a partial o-projection over its heads' slice of wo. Host sums the 4
group-partials per batch.

Device layouts are all "transposed" ([feature, seq]) so no on-device
transposes of activations are needed:
  - qT/kT: [hd, seq] with head-dim PERMUTED to [evens | odds] (the RoPE
    interleaved-pair rotation becomes two partition-aligned half-tiles).
    The permutation is folded into wq/wk columns host-side; scores are
    invariant since q and k are permuted consistently.
  - scoresT blocks: [sk=128, sq<=512] = kT_blk.T @ qT_tile (one matmul, K=hd).
    Diagonal-region blocks are narrowed to their causally-live sq range
    (min N=256 to stay at full float32r rate).
  - softmax without max-subtraction (|scores*scale| ~ O(10) in fp32 is safe);
    Z via ones-vector matmuls accumulated in PSUM; reciprocal batched [4,512]
    per q-tile; normalization via gpsimd partition-broadcast + DVE multiply.
  - AV: avT[hd, sq] += v_blk[sk, hd].T @ exp_blk[sk, sq]  (v natural layout,
    from vT via PE transposes, done per seq-tile so attention can start early).
  - o-proj interleaved per q-tile: out[sq,d] += outT_h[:, sq_blk].T @ wo[h_blk, d];
    partial written to HBM, summed on host across the 4 group cores per batch.

All matmuls run as float32r (full PE rate at N>=256, ~fp32 accuracy class).
"""

import numpy as np

B, S, D = 2, 2048, 2048
H, KV, HD = 16, 4, 128
GH = H // KV            # query heads per kv group (per core)
NCORES = 8
THETA = 10000.0
NEG = -1e9
SQT = 512               # q seq tile width
NSQ = S // SQT          # 4
NKB = S // 128          # 16 k blocks
NCH = D // 128          # 16 contraction chunks
NOFF = SQT // 128       # 4 diagonal offsets

SCALE = float(HD) ** -0.5

# Narrowed column start for diagonal-region block at offset o: the block is
# causally dead below sq_local = 128*o; keep N >= 256 for full f32r rate.
NARROW = [0, 128, 256, 256]
# Columns needing the 0/1 mask multiply (tri block + any dead cols kept wide).
MASKW = [(0, 128), (128, 256), (256, 384), (256, 512)]

# Exposed for the dev harness (test.py) to read profiling results.
last_results = None


def _build_program():
    from contextlib import ExitStack

    import concourse.tile as tile
    from concourse import bacc, mybir
    from concourse.masks import make_identity

    f32 = mybir.dt.float32
    f32r = mybir.dt.float32r
    EXP = mybir.ActivationFunctionType.Exp

    def r(ap):
        return ap.bitcast(f32r)

    nc = bacc.Bacc("TRN2", target_bir_lowering=False, debug=False,
                   num_devices=NCORES)

    # all bulk tensors are pre-rearranged on the host so every DMA is
    # contiguous per partition (HWDGE descriptor generation is ~7ns/descr
    # with a ~600ns floor: strided layouts cost ~100us of Sync-queue time)
    xT_d = nc.dram_tensor("xr", [128, NSQ, NCH, SQT], f32r, kind="ExternalInput")
    wq_d = nc.dram_tensor("wqp", [128, NCH, GH * HD], f32r, kind="ExternalInput")
    wk_d = nc.dram_tensor("wkp", [128, NCH, HD], f32r, kind="ExternalInput")
    wv_d = nc.dram_tensor("wvg", [128, NCH, HD], f32r, kind="ExternalInput")
    wo_d = nc.dram_tensor("wog", [128, GH, D], f32r, kind="ExternalInput")
    cos_d = nc.dram_tensor("cos2", [HD, S], f32, kind="ExternalInput")
    sin_d = nc.dram_tensor("sinS", [HD, S], f32, kind="ExternalInput")
    msk_d = nc.dram_tensor("m01", [128, NOFF, SQT], f32, kind="ExternalInput")
    one_d = nc.dram_tensor("ones1", [128, 1], f32r, kind="ExternalInput")
    out_d = nc.dram_tensor("out", [128, S // 128, D], f32, kind="ExternalOutput")

    xT_v = xT_d.ap()        # [128, NSQ, NCH, SQT]
    wq_v = wq_d.ap()
    wk_v = wk_d.ap()
    wv_v = wv_d.ap()
    wo_v = wo_d.ap()
    out_v = out_d.ap()      # [128, 16, 2048]; host untangles (m p) rows

    with tile.TileContext(nc) as tc, ExitStack() as ctx:
        persist = ctx.enter_context(tc.tile_pool(name="persist", bufs=1))

        qT = [persist.tile([128, S], f32r, name=f"qT{h}") for h in range(GH)]
        kT = persist.tile([128, S], f32r, name="kT")
        vn = persist.tile([128, NKB, HD], f32r, name="vn")
        cos2 = persist.tile([128, S], f32, name="cos2")
        sinS = persist.tile([128, S], f32, name="sinS")
        m01 = persist.tile([128, NOFF, SQT], f32, name="m01")
        ones = persist.tile([128, 1], f32r, name="ones")
        ident = persist.tile([128, 128], f32, name="ident")

        # aux loads on the ACT DMA queue; bulk data goes via Sync so the
        # first weight/x tiles are in flight immediately
        nc.scalar.dma_start(cos2[:], cos_d[:])
        nc.scalar.dma_start(sinS[:], sin_d[:])
        nc.scalar.dma_start(m01[:], msk_d[:])
        nc.scalar.dma_start(ones[:], one_d[:])
        make_identity(nc, ident[:])

        # ---------------- Phase 1: projections + RoPE + v ----------------
        with (
            tc.tile_pool(name="w1", bufs=1) as w1p,
            tc.tile_pool(name="xa", bufs=2) as xap,
            tc.tile_pool(name="raw", bufs=2) as rawp,
            tc.tile_pool(name="rope", bufs=2) as ropep,
            tc.tile_pool(name="ps1", bufs=1, space="PSUM") as ps1,
            tc.tile_pool(name="tps", bufs=2, space="PSUM") as tps,
        ):
            wq_sb = w1p.tile([128, NCH, GH * HD], f32r)
            wk_sb = w1p.tile([128, NCH, HD], f32r)
            wv_sb = w1p.tile([128, NCH, HD], f32r)
            vT_tmp = w1p.tile([128, S], f32)
            nc.sync.dma_start(wk_sb[:], wk_v)
            nc.sync.dma_start(wv_sb[:], wv_v)
            for cq in range(4):  # split so early q matmuls aren't gated
                nc.sync.dma_start(wq_sb[:, 4 * cq:4 * cq + 4, :],
                                  wq_v[:, 4 * cq:4 * cq + 4, :])

            def rope(raw, dst, t):
                """dst[:, t-tile] = rope(raw) in the [evens|odds] layout."""
                sl = np.s_[:, t * SQT:(t + 1) * SQT]
                tmp = ropep.tile([128, SQT], f32, tag="ropetmp", name="tmp")
                swp = ropep.tile([128, SQT], f32, tag="ropeswp", name="swp")
                nc.vector.tensor_mul(tmp[:], raw[:], cos2[sl])
                # swp[0:64] = odd*(-sin), swp[64:128] = even*(+sin); sinS is
                # stored [+sin | -sin] so each mul's two INPUTS share a base
                # partition (walrus requires that); only the output crosses.
                nc.vector.tensor_mul(swp[0:64, :], raw[64:128, :],
                                     sinS[sl][64:128, :])
                nc.vector.tensor_mul(swp[64:128, :], raw[0:64, :],
                                     sinS[sl][0:64, :])
                nc.vector.tensor_add(dst[sl], tmp[:], swp[:])

            for t in range(NSQ):
                ssl = np.s_[t * SQT:(t + 1) * SQT]
                q_ps = [ps1.tile([128, SQT], f32, tag=f"qps{h}", name=f"qps{h}")
                        for h in range(GH)]
                k_ps = ps1.tile([128, SQT], f32, tag="kps", name="k_ps")
                v_ps = ps1.tile([128, SQT], f32, tag="vps", name="v_ps")
                for ch in range(2):
                    xt = xap.tile([128, NCH // 2, SQT], f32r, tag="xt",
                                  name="xt")
                    nc.sync.dma_start(
                        xt[:], xT_v[:, t, 8 * ch:8 * ch + 8, :])
                    for c8 in range(NCH // 2):
                        c = 8 * ch + c8
                        st, sp = c == 0, c == NCH - 1
                        for h in range(GH):
                            nc.tensor.matmul(
                                q_ps[h][:], r(wq_sb[:, c, h * HD:(h + 1) * HD]),
                                r(xt[:, c8, :]), start=st, stop=sp)
                        nc.tensor.matmul(k_ps[:], r(wk_sb[:, c, :]),
                                         r(xt[:, c8, :]), start=st, stop=sp)
                        nc.tensor.matmul(v_ps[:], r(wv_sb[:, c, :]),
                                         r(xt[:, c8, :]), start=st, stop=sp)
                # psum -> sbuf copies split over ACT/DVE, rope on DVE
                for h in range(GH):
                    qraw = rawp.tile([128, SQT], f32, tag=f"qraw{h}",
                                     name=f"qraw{h}")
                    if h % 2 == 0:
                        nc.scalar.copy(qraw[:], q_ps[h][:])
                    else:
                        nc.vector.tensor_copy(qraw[:], q_ps[h][:])
                    rope(qraw, qT[h], t)
                kraw = rawp.tile([128, SQT], f32, tag="kraw", name="kraw")
                nc.scalar.copy(kraw[:], k_ps[:])
                rope(kraw, kT, t)
                nc.scalar.copy(vT_tmp[:, ssl], v_ps[:])
                # v natural blocks for this seq-tile (attention needs them
                # as soon as the t=0 slices exist)
                for j in range(NOFF * t, NOFF * (t + 1)):
                    t_ps = tps.tile([128, 128], f32, tag="tps", name="t_ps")
                    nc.tensor.transpose(
                        t_ps[:], vT_tmp[:, j * 128:(j + 1) * 128], ident[:])
                    nc.vector.tensor_copy(vn[:, j, :], t_ps[:])

        # -------- Phase 2: attention + o-projection, q-tile major --------
        with (
            tc.tile_pool(name="wo", bufs=1) as wop,
            tc.tile_pool(name="ot", bufs=1) as otp,
            tc.tile_pool(name="ex", bufs=6) as exp_p,
            tc.tile_pool(name="nrm", bufs=2) as nrm_p,
            tc.tile_pool(name="osb", bufs=2) as osb_p,
            tc.tile_pool(name="sps", bufs=3, space="PSUM") as sps_p,
            tc.tile_pool(name="avps", bufs=2, space="PSUM") as avp_p,
            tc.tile_pool(name="zps", bufs=1, space="PSUM") as zp_p,
            tc.tile_pool(name="ops", bufs=2, space="PSUM") as ops_p,
        ):
            wo_sb = wop.tile([128, GH, D], f32r)
            nc.scalar.dma_start(wo_sb[:], wo_v)
            outT = [otp.tile([128, S], f32r, name=f"outT{h}")
                    for h in range(GH)]

            for t in range(NSQ):
                qsl = np.s_[:, t * SQT:(t + 1) * SQT]
                nblk = NOFF * (t + 1)
                # head h's Z row lives at partition 32h (engine APs may only
                # start at partitions 0/32/64/96); unused rows memset to 1.0
                # so the batched reciprocal stays finite.
                zall = nrm_p.tile([128, SQT], f32, tag="zall", name="zall")
                nc.gpsimd.memset(zall[:], 1.0)
                av_list = []
                for h in range(GH):
                    av_ps = avp_p.tile([HD, SQT], f32, tag="av", name="av_ps")
                    z_ps = zp_p.tile([1, SQT], f32, tag="z", name="z_ps")
                    for j in range(nblk):
                        o = j - NOFF * t
                        lo = NARROW[o] if o >= 0 else 0
                        csl = np.s_[:, lo:SQT]
                        s_ps = sps_p.tile([128, SQT], f32, tag="s", name="s_ps")
                        nc.tensor.matmul(
                            s_ps[csl], r(kT[:, j * 128:(j + 1) * 128]),
                            r(qT[h][qsl][csl]))
                        e = exp_p.tile([128, SQT], f32r, tag="e", name="e")
                        nc.scalar.activation(e[csl], s_ps[csl], EXP,
                                             scale=SCALE)
                        if o >= 0:  # mask the causally-dead part
                            ma, mb = MASKW[o]
                            msl = np.s_[:, ma:mb]
                            nc.vector.tensor_mul(e[msl], e[msl],
                                                 m01[:, o, ma:mb])
                        st, sp = j == 0, j == nblk - 1
                        nc.tensor.matmul(av_ps[csl], r(vn[:, j, :]), r(e[csl]),
                                         start=st, stop=sp)
                        nc.tensor.matmul(z_ps[csl], r(ones[:]), r(e[csl]),
                                         start=st, stop=sp)
                    nc.vector.tensor_copy(zall[32 * h:32 * h + 1, :], z_ps[:])
                    # free the AV psum bank fast: unnormalized copy on ACT
                    avo = nrm_p.tile([HD, SQT], f32, tag=f"avo{h}",
                                     name=f"avo{h}", bufs=1)
                    nc.scalar.copy(avo[:], av_ps[:])
                    av_list.append(avo)
                zrec = nrm_p.tile([128, SQT], f32, tag="zrec", name="zrec")
                nc.vector.reciprocal(zrec[:], zall[:])
                for h in range(GH):
                    # stage to base partition 0: the HW broadcast ucode does
                    # not honor a nonzero AP base partition
                    z1 = nrm_p.tile([1, SQT], f32, tag="z1", name="z1")
                    nc.vector.tensor_copy(z1[:], zrec[32 * h:32 * h + 1, :])
                    zb = nrm_p.tile([128, SQT], f32, tag="zb", name="zb")
                    nc.gpsimd.partition_broadcast(zb[:], z1[:])
                    nc.vector.tensor_mul(outT[h][qsl], av_list[h][:], zb[:])

                # o-projection for this q-tile's four 128-row blocks;
                # one batched [128, D] store per row block
                for m in range(NOFF * t, NOFF * (t + 1)):
                    ob = osb_p.tile([128, D], f32, tag="ob", name="ob")
                    for jd in range(D // SQT):
                        dsl = np.s_[jd * SQT:(jd + 1) * SQT]
                        o_ps = ops_p.tile([128, SQT], f32, tag="o", name="o_ps")
                        for h in range(GH):
                            nc.tensor.matmul(
                                o_ps[:], r(outT[h][:, m * 128:(m + 1) * 128]),
                                r(wo_sb[:, h, dsl]),
                                start=(h == 0), stop=(h == GH - 1))
                        if (m + jd) % 2 == 0:
                            nc.vector.tensor_copy(ob[:, dsl], o_ps[:])
                        else:
                            nc.scalar.copy(ob[:, dsl], o_ps[:])
                    nc.sync.dma_start(out_v[:, m, :], ob[:])

    nc.compile()
    return nc


_prog = None


def _host_inputs(x, wq, wk, wv, wo):
    """Per-core input maps (core c -> batch c//KV, kv-group c%KV)."""
    perm = np.concatenate([np.arange(0, HD, 2), np.arange(1, HD, 2)])
    wq_p = np.ascontiguousarray(
        wq.reshape(D, H, HD)[:, :, perm].reshape(D, H * HD))
    wk_p = np.ascontiguousarray(
        wk.reshape(D, KV, HD)[:, :, perm].reshape(D, KV * HD))

    inv_freq = 1.0 / (THETA ** (np.arange(0, HD, 2, dtype=np.float64) / HD))
    freqs = np.outer(np.arange(S, dtype=np.float64), inv_freq)   # [S, 64]
    cosT = np.cos(freqs).T.astype(np.float32)                    # [64, S]
    sinT = np.sin(freqs).T.astype(np.float32)
    cos2 = np.ascontiguousarray(np.concatenate([cosT, cosT], 0))
    sinS = np.ascontiguousarray(np.concatenate([sinT, -sinT], 0))

    sk = np.arange(128)[:, None]
    sq = np.arange(SQT)[None, :]
    m01 = np.stack([(sk <= sq - 128 * o).astype(np.float32)
                    for o in range(NOFF)], axis=1)               # [128,4,512]
    m01 = np.ascontiguousarray(m01)

    def part_major(a):
        """[NCH*128, W] -> [128, NCH, W] (chunk-of-contraction per partition)."""
        w = a.shape[1]
        return np.ascontiguousarray(
            a.reshape(NCH, 128, w).transpose(1, 0, 2))

    in_maps = []
    for c in range(NCORES):
        b, g = c // KV, c % KV
        xr = np.ascontiguousarray(
            x[b].T.reshape(NCH, 128, NSQ, SQT).transpose(1, 2, 0, 3))
        wog = np.ascontiguousarray(
            wo[g * GH * HD:(g + 1) * GH * HD, :].reshape(GH, 128, D)
            .transpose(1, 0, 2))
        in_maps.append({
            "xr": xr,
            "wqp": part_major(wq_p[:, g * GH * HD:(g + 1) * GH * HD]),
            "wkp": part_major(wk_p[:, g * HD:(g + 1) * HD]),
            "wvg": part_major(wv[:, g * HD:(g + 1) * HD]),
            "wog": wog,
            "cos2": cos2,
            "sinS": sinS,
            "m01": m01,
            "ones1": np.ones((128, 1), np.float32),
        })
    return in_maps


def _numpy_reference(x, mask, wq, wk, wv, wo):
    """Pure-numpy fallback for inputs this kernel isn't specialized for."""
    b, s, _ = x.shape
    q = (x @ wq).reshape(b, s, H, HD)
    k = (x @ wk).reshape(b, s, KV, HD)
    v = (x @ wv).reshape(b, s, KV, HD)
    inv_freq = 1.0 / (THETA ** (np.arange(0, HD, 2, dtype=np.float32) / HD))
    t = np.arange(s, dtype=np.float32)
    freqs = np.outer(t, inv_freq)
    cos = np.cos(freqs)[:, None, :]
    sin = np.sin(freqs)[:, None, :]

    def rot(a):
        bb, ss, nh, hd = a.shape
        a = a.reshape(bb, ss, nh, hd // 2, 2)
        a0, a1 = a[..., 0], a[..., 1]
        out = np.stack([a0 * cos - a1 * sin, a0 * sin + a1 * cos], axis=-1)
        return out.reshape(bb, ss, nh, hd)

    q, k = rot(q), rot(k)
    rep = H // KV
    k = np.repeat(k, rep, axis=2)
    v = np.repeat(v, rep, axis=2)
    q, k, v = (a.transpose(0, 2, 1, 3) for a in (q, k, v))
    scores = np.einsum("bhqd,bhkd->bhqk", q, k) * SCALE + mask
    scores = scores - scores.max(axis=-1, keepdims=True)
    e = np.exp(scores)
    attn = e / e.sum(axis=-1, keepdims=True)
    out = np.einsum("bhqk,bhkd->bhqd", attn, v)
    out = out.transpose(0, 2, 1, 3).reshape(b, s, H * HD)
    return (out @ wo).astype(np.float32)


def kernel(x, mask, wq, wk, wv, wo):
    global _prog, last_results
    x = np.asarray(x, np.float32)
    mask = np.asarray(mask, np.float32)
    wq, wk, wv, wo = (np.asarray(a, np.float32) for a in (wq, wk, wv, wo))

    causal = np.where(np.tril(np.ones((S, S), bool)), 0.0, NEG).astype(np.float32)
    if (x.shape != (B, S, D) or mask.shape != (S, S)
            or not np.array_equal(mask, causal)):
        return _numpy_reference(x, mask, wq, wk, wv, wo)

    from concourse import bass_utils

    if _prog is None:
        _prog = _build_program()

    in_maps = _host_inputs(x, wq, wk, wv, wo)
    last_results = bass_utils.run_bass_kernel_spmd(
        _prog, in_maps, core_ids=list(range(NCORES)))
    # device layout [128, 16, D]: logical row = m*128 + p
    parts = [res["out"].transpose(1, 0, 2).reshape(S, D)
             for res in last_results.results]
    out = np.empty((B, S, D), np.float32)
    for b in range(B):
        out[b] = parts[KV * b] + parts[KV * b + 1] + parts[KV * b + 2] + parts[KV * b + 3]
    return out


# revision 16
# speedup vs baseline: 1.2572x; 1.0766x over previous
"""GQA causal attention (B=2,S=2048,D=2048,H=16,KV=4,HD=128) on 8 TRN2 NeuronCores.

Sharding: core c handles (batch b=c//4, kv-group g=c%4) — exactly 8 shards.
Each core computes q/k/v projections for its group's 4 query heads + 1 kv head,
RoPE, causal attention (512-wide q tiles, skipping fully-masked k blocks),
and a partial o-projection over its heads' slice of wo. Host sums the 4
group-partials per batch.

Device layouts are all "transposed" ([feature, seq]) so no on-device
transposes of activations are needed:
  - qT/kT: [hd, seq] with head-dim PERMUTED to [evens | odds] (the RoPE
    interleaved-pair rotation becomes two partition-aligned half-tiles).
    The permutation is folded into wq/wk columns host-side; scores are
    invariant since q and k are permuted consistently.
  - scoresT blocks: [sk=128, sq<=512] = kT_blk.T @ qT_tile (one matmul, K=hd).
    Diagonal-region blocks are narrowed to their causally-live sq range
    (min N=256 to stay at full float32r rate).
  - softmax without max-subtraction (|scores*scale| ~ O(10) in fp32 is safe);
    Z via ones-vector matmuls accumulated in PSUM; reciprocal batched [4,512]
    per q-tile; normalization via gpsimd partition-broadcast + DVE multiply.
  - AV: avT[hd, sq] += v_blk[sk, hd].T @ exp_blk[sk, sq]  (v natural layout,
    from vT via PE transposes, done per seq-tile so attention can start early).
  - o-proj interleaved per q-tile: out[sq,d] += outT_h[:, sq_blk].T @ wo[h_blk, d];
    partial written to HBM, summed on host across the 4 group cores per batch.

All matmuls run as float32r (full PE rate at N>=256, ~fp32 accuracy class).
"""

import numpy as np

B, S, D = 2, 2048, 2048
H, KV, HD = 16, 4, 128
GH = H // KV            # query heads per kv group (per core)
NCORES = 8
THETA = 10000.0
NEG = -1e9
SQT = 512               # q seq tile width
NSQ = S // SQT          # 4
NKB = S // 128          # 16 k blocks
NCH = D // 128          # 16 contraction chunks
NOFF = SQT // 128       # 4 diagonal offsets

SCALE = float(HD) ** -0.5

# Narrowed column start for diagonal-region block at offset o: the block is
# causally dead below sq_local = 128*o; keep N >= 256 for full f32r rate.
NARROW = [0, 128, 256, 256]
# Columns needing the 0/1 mask multiply (tri block + any dead cols kept wide).
MASKW = [(0, 128), (128, 256), (256, 384), (256, 512)]

# Exposed for the dev harness (test.py) to read profiling results.
last_results = None


def _build_program():
    from contextlib import ExitStack

    import concourse.tile as tile
    from concourse import bacc, mybir
    from concourse.masks import make_identity

    f32 = mybir.dt.float32
    f32r = mybir.dt.float32r
    EXP = mybir.ActivationFunctionType.Exp

    def r(ap):
        return ap.bitcast(f32r)

    nc = bacc.Bacc("TRN2", target_bir_lowering=False, debug=False,
                   num_devices=NCORES)

    # all bulk tensors are pre-rearranged on the host so every DMA is
    # contiguous per partition (HWDGE descriptor generation is ~7ns/descr
    # with a ~600ns floor: strided layouts cost ~100us of Sync-queue time)
    xT_d = nc.dram_tensor("xr", [128, NSQ, NCH, SQT], f32r, kind="ExternalInput")
    wq_d = nc.dram_tensor("wqp", [128, NCH, GH * HD], f32r, kind="ExternalInput")
    wk_d = nc.dram_tensor("wkp", [128, NCH, HD], f32r, kind="ExternalInput")
    wv_d = nc.dram_tensor("wvg", [128, NCH, HD], f32r, kind="ExternalInput")
    wo_d = nc.dram_tensor("wog", [128, GH, D], f32r, kind="ExternalInput")
    cos_d = nc.dram_tensor("cos2", [HD, S], f32, kind="ExternalInput")
    sin_d = nc.dram_tensor("sinS", [HD, S], f32, kind="ExternalInput")
    msk_d = nc.dram_tensor("m01", [128, NOFF, SQT], f32, kind="ExternalInput")
    one_d = nc.dram_tensor("ones1", [128, 1], f32r, kind="ExternalInput")
    out_d = nc.dram_tensor("out", [128, S // 128, D], f32, kind="ExternalOutput")

    xT_v = xT_d.ap()        # [128, NSQ, NCH, SQT]
    wq_v = wq_d.ap()
    wk_v = wk_d.ap()
    wv_v = wv_d.ap()
    wo_v = wo_d.ap()
    out_v = out_d.ap()      # [128, 16, 2048]; host untangles (m p) rows

    with tile.TileContext(nc) as tc, ExitStack() as ctx:
        persist = ctx.enter_context(tc.tile_pool(name="persist", bufs=1))

        qT = [persist.tile([128, S], f32r, name=f"qT{h}") for h in range(GH)]
        kT = persist.tile([128, S], f32r, name="kT")
        vn = persist.tile([128, NKB, HD], f32r, name="vn")
        cos2 = persist.tile([128, S], f32, name="cos2")
        sinS = persist.tile([128, S], f32, name="sinS")
        m01 = persist.tile([128, NOFF, SQT], f32, name="m01")
        ones = persist.tile([128, 1], f32r, name="ones")
        ident = persist.tile([128, 128], f32, name="ident")

        # aux loads off the Sync queue so the first x/weight tiles are in
        # flight immediately
        nc.gpsimd.dma_start(ones[:], one_d[:])
        nc.scalar.dma_start(cos2[:], cos_d[:])
        nc.scalar.dma_start(sinS[:], sin_d[:])
        nc.scalar.dma_start(m01[:], msk_d[:])
        make_identity(nc, ident[:])
        # dummy broadcast: loads the gpsimd ucode overlay (~10us) off the
        # critical path — the first real one otherwise stalls t=0 normalize
        warm = persist.tile([128, 1], f32, name="warm")
        nc.gpsimd.partition_broadcast(warm[:], ones[0:1, :].bitcast(f32))

        # ---------------- Phase 1: projections + RoPE + v ----------------
        with (
            tc.tile_pool(name="w1", bufs=1) as w1p,
            tc.tile_pool(name="xa", bufs=2) as xap,
            tc.tile_pool(name="raw", bufs=2) as rawp,
            tc.tile_pool(name="rope", bufs=2) as ropep,
            tc.tile_pool(name="ps1", bufs=1, space="PSUM") as ps1,
            tc.tile_pool(name="tps", bufs=2, space="PSUM") as tps,
        ):
            wq_sb = w1p.tile([128, NCH, GH * HD], f32r)
            wk_sb = w1p.tile([128, NCH, HD], f32r)
            wv_sb = w1p.tile([128, NCH, HD], f32r)
            vT_tmp = w1p.tile([128, S], f32)
            nc.gpsimd.dma_start(wk_sb[:], wk_v)
            nc.gpsimd.dma_start(wv_sb[:], wv_v)
            for cq in range(4):  # split so early q matmuls aren't gated
                nc.scalar.dma_start(wq_sb[:, 4 * cq:4 * cq + 4, :],
                                    wq_v[:, 4 * cq:4 * cq + 4, :])

            def rope(raw, dst, t):
                """dst[:, t-tile] = rope(raw) in the [evens|odds] layout."""
                sl = np.s_[:, t * SQT:(t + 1) * SQT]
                tmp = ropep.tile([128, SQT], f32, tag="ropetmp", name="tmp")
                swp = ropep.tile([128, SQT], f32, tag="ropeswp", name="swp")
                nc.vector.tensor_mul(tmp[:], raw[:], cos2[sl])
                # swp[0:64] = odd*(-sin), swp[64:128] = even*(+sin); sinS is
                # stored [+sin | -sin] so each mul's two INPUTS share a base
                # partition (walrus requires that); only the output crosses.
                nc.vector.tensor_mul(swp[0:64, :], raw[64:128, :],
                                     sinS[sl][64:128, :])
                nc.vector.tensor_mul(swp[64:128, :], raw[0:64, :],
                                     sinS[sl][0:64, :])
                nc.vector.tensor_add(dst[sl], tmp[:], swp[:])

            for t in range(NSQ):
                ssl = np.s_[t * SQT:(t + 1) * SQT]
                q_ps = [ps1.tile([128, SQT], f32, tag=f"qps{h}", name=f"qps{h}")
                        for h in range(GH)]
                k_ps = ps1.tile([128, SQT], f32, tag="kps", name="k_ps")
                v_ps = ps1.tile([128, SQT], f32, tag="vps", name="v_ps")
                for ch in range(2):
                    xt = xap.tile([128, NCH // 2, SQT], f32r, tag="xt",
                                  name="xt")
                    eng = nc.sync if ch == 0 else nc.gpsimd
                    if t == 0 and ch == 0:
                        # split the very first tile across two queues so the
                        # first matmul's data lands in ~half the time
                        nc.sync.dma_start(xt[:, 0:4, :],
                                          xT_v[:, t, 0:4, :])
                        nc.gpsimd.dma_start(xt[:, 4:8, :],
                                            xT_v[:, t, 4:8, :])
                    else:
                        eng.dma_start(xt[:], xT_v[:, t, 8 * ch:8 * ch + 8, :])
                    for c8 in range(NCH // 2):
                        c = 8 * ch + c8
                        st, sp = c == 0, c == NCH - 1
                        for h in range(GH):
                            nc.tensor.matmul(
                                q_ps[h][:], r(wq_sb[:, c, h * HD:(h + 1) * HD]),
                                r(xt[:, c8, :]), start=st, stop=sp)
                        nc.tensor.matmul(k_ps[:], r(wk_sb[:, c, :]),
                                         r(xt[:, c8, :]), start=st, stop=sp)
                        nc.tensor.matmul(v_ps[:], r(wv_sb[:, c, :]),
                                         r(xt[:, c8, :]), start=st, stop=sp)
                # psum -> sbuf copies split over ACT/DVE; transposes and
                # their vn copies come BEFORE rope so the DVE queue doesn't
                # block the PE on freeing transpose psum slots
                nc.scalar.copy(vT_tmp[:, ssl], v_ps[:])
                for j in range(NOFF * t, NOFF * (t + 1)):
                    t_ps = tps.tile([128, 128], f32, tag="tps", name="t_ps")
                    nc.tensor.transpose(
                        t_ps[:], vT_tmp[:, j * 128:(j + 1) * 128], ident[:])
                    if j % 2 == 0:
                        nc.vector.tensor_copy(vn[:, j, :], t_ps[:])
                    else:
                        nc.scalar.copy(vn[:, j, :], t_ps[:])
                for h in range(GH):
                    qraw = rawp.tile([128, SQT], f32, tag=f"qraw{h}",
                                     name=f"qraw{h}")
                    if h % 2 == 0:
                        nc.scalar.copy(qraw[:], q_ps[h][:])
                    else:
                        nc.vector.tensor_copy(qraw[:], q_ps[h][:])
                    rope(qraw, qT[h], t)
                kraw = rawp.tile([128, SQT], f32, tag="kraw", name="kraw")
                nc.vector.tensor_copy(kraw[:], k_ps[:])
                rope(kraw, kT, t)

        # -------- Phase 2: attention + o-projection, q-tile major --------
        with (
            tc.tile_pool(name="wo", bufs=1) as wop,
            tc.tile_pool(name="ot", bufs=1) as otp,
            tc.tile_pool(name="ex", bufs=6) as exp_p,
            tc.tile_pool(name="nrm", bufs=2) as nrm_p,
            tc.tile_pool(name="osb", bufs=2) as osb_p,
            tc.tile_pool(name="sps", bufs=3, space="PSUM") as sps_p,
            tc.tile_pool(name="avps", bufs=2, space="PSUM") as avp_p,
            tc.tile_pool(name="zps", bufs=1, space="PSUM") as zp_p,
            tc.tile_pool(name="ops", bufs=2, space="PSUM") as ops_p,
        ):
            wo_sb = wop.tile([128, GH, D], f32r)
            nc.scalar.dma_start(wo_sb[:], wo_v)
            outT = [otp.tile([128, S], f32r, name=f"outT{h}")
                    for h in range(GH)]

            def o_proj(t):
                # o-projection for q-tile t's four 128-row blocks; one
                # batched [128, D] store per row block
                for m in range(NOFF * t, NOFF * (t + 1)):
                    ob = osb_p.tile([128, D], f32, tag="ob", name="ob")
                    for jd in range(D // SQT):
                        dsl = np.s_[jd * SQT:(jd + 1) * SQT]
                        o_ps = ops_p.tile([128, SQT], f32, tag="o", name="o_ps")
                        for h in range(GH):
                            nc.tensor.matmul(
                                o_ps[:], r(outT[h][:, m * 128:(m + 1) * 128]),
                                r(wo_sb[:, h, dsl]),
                                start=(h == 0), stop=(h == GH - 1))
                        if (m + jd) % 2 == 0:
                            nc.vector.tensor_copy(ob[:, dsl], o_ps[:])
                        else:
                            nc.scalar.copy(ob[:, dsl], o_ps[:])
                    nc.sync.dma_start(out_v[:, m, :], ob[:])

            for t in range(NSQ):
                qsl = np.s_[:, t * SQT:(t + 1) * SQT]
                nblk = NOFF * (t + 1)
                # head h's Z row lives at partition 32h (engine APs may only
                # start at partitions 0/32/64/96); unused rows memset to 1.0
                # so the batched reciprocal stays finite.
                zall = nrm_p.tile([128, SQT], f32, tag="zall", name="zall")
                nc.gpsimd.memset(zall[:], 1.0)
                av_list = []
                for h in range(GH):
                    av_ps = avp_p.tile([HD, SQT], f32, tag="av", name="av_ps")
                    z_ps = zp_p.tile([1, SQT], f32, tag="z", name="z_ps")
                    for j in range(nblk):
                        o = j - NOFF * t
                        lo = NARROW[o] if o >= 0 else 0
                        csl = np.s_[:, lo:SQT]
                        s_ps = sps_p.tile([128, SQT], f32, tag="s", name="s_ps")
                        nc.tensor.matmul(
                            s_ps[csl], r(kT[:, j * 128:(j + 1) * 128]),
                            r(qT[h][qsl][csl]))
                        e = exp_p.tile([128, SQT], f32r, tag="e", name="e")
                        nc.scalar.activation(e[csl], s_ps[csl], EXP,
                                             scale=SCALE)
                        if o >= 0:  # mask the causally-dead part
                            ma, mb = MASKW[o]
                            msl = np.s_[:, ma:mb]
                            nc.vector.tensor_mul(e[msl], e[msl],
                                                 m01[:, o, ma:mb])
                        st, sp = j == 0, j == nblk - 1
                        nc.tensor.matmul(av_ps[csl], r(vn[:, j, :]), r(e[csl]),
                                         start=st, stop=sp)
                        nc.tensor.matmul(z_ps[csl], r(ones[:]), r(e[csl]),
                                         start=st, stop=sp)
                    nc.vector.tensor_copy(zall[32 * h:32 * h + 1, :], z_ps[:])
                    # free the AV psum bank fast: unnormalized copy on ACT
                    avo = nrm_p.tile([HD, SQT], f32, tag=f"avo{h}",
                                     name=f"avo{h}", bufs=1)
                    nc.scalar.copy(avo[:], av_ps[:])
                    av_list.append(avo)
                zrec = nrm_p.tile([128, SQT], f32, tag="zrec", name="zrec")
                nc.vector.reciprocal(zrec[:], zall[:])
                for h in range(GH):
                    # stage to base partition 0: the HW broadcast ucode does
                    # not honor a nonzero AP base partition
                    z1 = nrm_p.tile([1, SQT], f32, tag="z1", name="z1")
                    nc.vector.tensor_copy(z1[:], zrec[32 * h:32 * h + 1, :])
                    zb = nrm_p.tile([128, SQT], f32, tag="zb", name="zb")
                    nc.gpsimd.partition_broadcast(zb[:], z1[:])
                    nc.vector.tensor_mul(outT[h][qsl], av_list[h][:], zb[:])

                # o-projection pipelined one q-tile behind: the normalize
                # chain of tile t completes under tile t+1's attention
                if t > 0:
                    o_proj(t - 1)
            o_proj(NSQ - 1)

    nc.compile()
    return nc


_prog = None


def _host_inputs(x, wq, wk, wv, wo):
    """Per-core input maps (core c -> batch c//KV, kv-group c%KV)."""
    perm = np.concatenate([np.arange(0, HD, 2), np.arange(1, HD, 2)])
    wq_p = np.ascontiguousarray(
        wq.reshape(D, H, HD)[:, :, perm].reshape(D, H * HD))
    wk_p = np.ascontiguousarray(
        wk.reshape(D, KV, HD)[:, :, perm].reshape(D, KV * HD))

    inv_freq = 1.0 / (THETA ** (np.arange(0, HD, 2, dtype=np.float64) / HD))
    freqs = np.outer(np.arange(S, dtype=np.float64), inv_freq)   # [S, 64]
    cosT = np.cos(freqs).T.astype(np.float32)                    # [64, S]
    sinT = np.sin(freqs).T.astype(np.float32)
    cos2 = np.ascontiguousarray(np.concatenate([cosT, cosT], 0))
    sinS = np.ascontiguousarray(np.concatenate([sinT, -sinT], 0))

    sk = np.arange(128)[:, None]
    sq = np.arange(SQT)[None, :]
    m01 = np.stack([(sk <= sq - 128 * o).astype(np.float32)
                    for o in range(NOFF)], axis=1)               # [128,4,512]
    m01 = np.ascontiguousarray(m01)

    def part_major(a):
        """[NCH*128, W] -> [128, NCH, W] (chunk-of-contraction per partition)."""
        w = a.shape[1]
        return np.ascontiguousarray(
            a.reshape(NCH, 128, w).transpose(1, 0, 2))

    in_maps = []
    for c in range(NCORES):
        b, g = c // KV, c % KV
        xr = np.ascontiguousarray(
            x[b].T.reshape(NCH, 128, NSQ, SQT).transpose(1, 2, 0, 3))
        wog = np.ascontiguousarray(
            wo[g * GH * HD:(g + 1) * GH * HD, :].reshape(GH, 128, D)
            .transpose(1, 0, 2))
        in_maps.append({
            "xr": xr,
            "wqp": part_major(wq_p[:, g * GH * HD:(g + 1) * GH * HD]),
            "wkp": part_major(wk_p[:, g * HD:(g + 1) * HD]),
            "wvg": part_major(wv[:, g * HD:(g + 1) * HD]),
            "wog": wog,
            "cos2": cos2,
            "sinS": sinS,
            "m01": m01,
            "ones1": np.ones((128, 1), np.float32),
        })
    return in_maps


def _numpy_reference(x, mask, wq, wk, wv, wo):
    """Pure-numpy fallback for inputs this kernel isn't specialized for."""
    b, s, _ = x.shape
    q = (x @ wq).reshape(b, s, H, HD)
    k = (x @ wk).reshape(b, s, KV, HD)
    v = (x @ wv).reshape(b, s, KV, HD)
    inv_freq = 1.0 / (THETA ** (np.arange(0, HD, 2, dtype=np.float32) / HD))
    t = np.arange(s, dtype=np.float32)
    freqs = np.outer(t, inv_freq)
    cos = np.cos(freqs)[:, None, :]
    sin = np.sin(freqs)[:, None, :]

    def rot(a):
        bb, ss, nh, hd = a.shape
        a = a.reshape(bb, ss, nh, hd // 2, 2)
        a0, a1 = a[..., 0], a[..., 1]
        out = np.stack([a0 * cos - a1 * sin, a0 * sin + a1 * cos], axis=-1)
        return out.reshape(bb, ss, nh, hd)

    q, k = rot(q), rot(k)
    rep = H // KV
    k = np.repeat(k, rep, axis=2)
    v = np.repeat(v, rep, axis=2)
    q, k, v = (a.transpose(0, 2, 1, 3) for a in (q, k, v))
    scores = np.einsum("bhqd,bhkd->bhqk", q, k) * SCALE + mask
    scores = scores - scores.max(axis=-1, keepdims=True)
    e = np.exp(scores)
    attn = e / e.sum(axis=-1, keepdims=True)
    out = np.einsum("bhqk,bhkd->bhqd", attn, v)
    out = out.transpose(0, 2, 1, 3).reshape(b, s, H * HD)
    return (out @ wo).astype(np.float32)


def kernel(x, mask, wq, wk, wv, wo):
    global _prog, last_results
    x = np.asarray(x, np.float32)
    mask = np.asarray(mask, np.float32)
    wq, wk, wv, wo = (np.asarray(a, np.float32) for a in (wq, wk, wv, wo))

    causal = np.where(np.tril(np.ones((S, S), bool)), 0.0, NEG).astype(np.float32)
    if (x.shape != (B, S, D) or mask.shape != (S, S)
            or not np.array_equal(mask, causal)):
        return _numpy_reference(x, mask, wq, wk, wv, wo)

    from concourse import bass_utils

    if _prog is None:
        _prog = _build_program()

    in_maps = _host_inputs(x, wq, wk, wv, wo)
    last_results = bass_utils.run_bass_kernel_spmd(
        _prog, in_maps, core_ids=list(range(NCORES)))
    # device layout [128, 16, D]: logical row = m*128 + p
    parts = [res["out"].transpose(1, 0, 2).reshape(S, D)
             for res in last_results.results]
    out = np.empty((B, S, D), np.float32)
    for b in range(B):
        out[b] = parts[KV * b] + parts[KV * b + 1] + parts[KV * b + 2] + parts[KV * b + 3]
    return out
